# revision 5
# baseline (speedup 1.0000x reference)
"""Trainium2 Bass kernel: dual-stream multi-branch attention (batch-parallel over 8 cores).

Reference computation (per batch element b):
    q  = x @ Wq.T + bq ; k = x @ Wk.T + bk ; v = x @ Wv.T + bv      (x stream)
    qd = y @ Wqd.T+ bqd; kd = y @ Wkd.T+bkd; vd = y @ Wvd.T+bvd     (y stream)
    4 attention branches over 12 heads (DH=64):
        b1 = attn(q, k, v)    -> stream-x self    (weight w11, proj Wo)
        b2 = attn(qd, kd, vd) -> stream-y self    (w21, Wod)
        b3 = attn(q, kd, vd)  -> stream-x cross   (w12, Wo)
        b4 = attn(qd, k, v)   -> stream-y cross   (w22, Wod)
    out_x = (w11*ctx1 + w12*ctx3) @ Wo.T  + (w11+w12)*bo
    out_y = (w21*ctx2 + w22*ctx4) @ Wod.T + (w21+w22)*bod

Device design (per core = one batch element, SPMD on 8 cores, no collectives):
  - Activations kept TRANSPOSED in SBUF: xT/yT/QT/KT [768(e-chunks of 128 partitions), 1024].
  - V kept NATURAL [s, e], augmented with a ones column per head -> softmax
    denominators come free as row 64 of the ctx matmul output.
  - scoresT[s2, s1] = k_h @ q_h.T via matmul(lhsT=kT_h, rhs=qT_h); exp on ACT
    (scale=1/8 folded into ACT's free affine); ctxT[d, s1] accumulated over s2
    chunks with lhsT = [v_h | ones].
  - Normalization: rr = 1/sums broadcast over partitions (gpsimd), fused
    scale+mul on DVE; merged per stream into CTXm.T (bf16) = out-proj rhs.
  - All matmul operands bf16 (PSUM accumulates f32).
"""

import numpy as np
import ml_dtypes
from contextlib import ExitStack

import concourse.bass as bass
import concourse.mybir as mybir
import concourse.tile as tile
from concourse import bacc
from concourse.alu_op_type import AluOpType
from concourse.bass_utils import run_bass_kernel_spmd

B, S, H = 8, 1024, 768
NH, DH = 12, 64
HC = H // 128          # 6 chunks of 128 along hidden dim
SC = S // 128          # 8 chunks of 128 along sequence dim
N_CORES = 8
BF16 = mybir.dt.bfloat16
F32 = mybir.dt.float32
EXPFN = mybir.ActivationFunctionType.Exp


def _build(scalars, bias_flags):
    """Build the Bass program. scalars = (w11, w12, w21, w22) python floats
    (baked as immediates). bias_flags: dict name->bool for nonzero biases."""
    w11, w12, w21, w22 = scalars
    nc = bacc.Bacc(debug=False)

    # ---- DRAM params (per-core shard views; weights replicated) ----
    def din(name, shape, dt=BF16):
        return nc.declare_dram_parameter(name, list(shape), dt, isOutput=False)

    xT_d = din("xT", (H, S))
    yT_d = din("yT", (H, S))
    wd = {n: din(n, (H, H)) for n in
          ("WqT", "WkT", "WvT", "WqdT", "WkdT", "WvdT", "WoT", "WodT")}
    bias_d = {}
    for n in ("bq", "bk", "bqd", "bkd", "bfx", "bfy"):
        if bias_flags.get(n):
            bias_d[n] = din(n, (H,), F32)
    out_d = nc.declare_dram_parameter("out", [2, H, S], F32, isOutput=True)

    def wdram(n):
        # [768, 768] -> [128 partitions, chunk, 768]
        return wd[n].rearrange("(c p) e -> p c e", p=128)

    with tile.TileContext(nc) as tc, ExitStack() as top:
        qk_pool = top.enter_context(tc.tile_pool(name="qk", bufs=1))
        v_pool = top.enter_context(tc.tile_pool(name="vv", bufs=1))
        ctxm_pool = top.enter_context(tc.tile_pool(name="ctxm", bufs=1))
        bias_pool = top.enter_context(tc.tile_pool(name="bias", bufs=1))

        QT = qk_pool.tile([128, HC, S], BF16, tag="QT")
        KT = qk_pool.tile([128, HC, S], BF16, tag="KT")
        QdT = qk_pool.tile([128, HC, S], BF16, tag="QdT")
        KdT = qk_pool.tile([128, HC, S], BF16, tag="KdT")
        Vp = v_pool.tile([128, SC, NH, DH + 1], BF16, tag="Vp")
        Vdp = v_pool.tile([128, SC, NH, DH + 1], BF16, tag="Vdp")
        CX = ctxm_pool.tile([128, HC, S], BF16, tag="CX")
        CY = ctxm_pool.tile([128, HC, S], BF16, tag="CY")

        # ones columns for softmax denominators
        nc.gpsimd.memset(Vp[:, :, :, DH:DH + 1], 1.0)
        nc.gpsimd.memset(Vdp[:, :, :, DH:DH + 1], 1.0)

        # per-partition bias tiles [128, HC]
        bias_t = {}
        for n in ("bq", "bk", "bqd", "bkd", "bfx", "bfy"):
            if n in bias_d:
                t = bias_pool.tile([128, HC], F32, tag=n)
                nc.sync.dma_start(t, bias_d[n].rearrange("(c p) -> p c", p=128))
                bias_t[n] = t

        # ---------------- Phase 1: projections ----------------
        with ExitStack() as ph:
            xy = ph.enter_context(tc.tile_pool(name="xy", bufs=1))
            wrot = ph.enter_context(tc.tile_pool(name="wrot", bufs=2))
            pjq = ph.enter_context(tc.tile_pool(name="pjq", bufs=4, space="PSUM"))
            pjv = ph.enter_context(tc.tile_pool(name="pjv", bufs=4, space="PSUM"))

            xT = xy.tile([128, HC, S], BF16, tag="xT")
            yT = xy.tile([128, HC, S], BF16, tag="yT")
            nc.sync.dma_start(xT, xT_d.rearrange("(c p) s -> p c s", p=128))
            nc.sync.dma_start(yT, yT_d.rearrange("(c p) s -> p c s", p=128))

            # transposed projections: QT = WqT.T @ xT (chunked)
            for wname, dst, act, bn in (
                ("WqT", QT, xT, "bq"), ("WkT", KT, xT, "bk"),
                ("WqdT", QdT, yT, "bqd"), ("WkdT", KdT, yT, "bkd"),
            ):
                W = wrot.tile([128, HC, H], BF16, tag="W")
                nc.sync.dma_start(W, wdram(wname))
                for m in range(HC):
                    for sn in range(2):
                        ps = pjq.tile([128, 512], F32, tag="pq")
                        for c in range(HC):
                            nc.tensor.matmul(
                                ps,
                                W[:, c, m * 128:(m + 1) * 128],
                                act[:, c, sn * 512:(sn + 1) * 512],
                                start=(c == 0), stop=(c == HC - 1),
                            )
                        dslice = dst[:, m, sn * 512:(sn + 1) * 512]
                        if bn in bias_t:
                            nc.vector.tensor_scalar_add(
                                dslice, ps, bias_t[bn][:, m:m + 1])
                        else:
                            nc.vector.tensor_copy(dslice, ps)

            # natural-layout V projections into augmented tiles
            for wname, dst, act in (("WvT", Vp, xT), ("WvdT", Vdp, yT)):
                W = wrot.tile([128, HC, H], BF16, tag="W")
                nc.sync.dma_start(W, wdram(wname))
                for s in range(SC):
                    for en in range(2):   # halves of 6 heads (384 cols)
                        ps = pjv.tile([128, 6, DH], F32, tag="pv")
                        for c in range(HC):
                            nc.tensor.matmul(
                                ps,
                                act[:, c, s * 128:(s + 1) * 128],
                                W[:, c, en * 384:(en + 1) * 384],
                                start=(c == 0), stop=(c == HC - 1),
                            )
                        nc.vector.tensor_copy(
                            dst[:, s, en * 6:(en + 1) * 6, 0:DH], ps)

        # ---------------- Phase 2: attention ----------------
        with ExitStack() as ph:
            sc_ps = ph.enter_context(tc.tile_pool(name="scps", bufs=2, space="PSUM"))
            cx_ps = ph.enter_context(tc.tile_pool(name="cxps", bufs=2, space="PSUM"))
            ex_pool = ph.enter_context(tc.tile_pool(name="exp", bufs=3))
            sm_pool = ph.enter_context(tc.tile_pool(name="small", bufs=2))
            rr_pool = ph.enter_context(tc.tile_pool(name="rrp", bufs=3))
            t_pool = ph.enter_context(tc.tile_pool(name="tmrg", bufs=4))

            def branch(kT, qT, Vaug, h, wscale):
                """One attention branch for head h. Returns normalized+scaled
                ctxT [64, S] f32 in SBUF (t = wscale * ctxu/sums)."""
                po = 64 * (h % 2)
                m = h // 2
                expt = ex_pool.tile([128, SC, S], BF16, tag="exp")
                for c2 in range(SC):
                    ps = sc_ps.tile([128, S], F32, tag="sc")
                    for hf in range(2):
                        nc.tensor.matmul(
                            ps[:, hf * 512:(hf + 1) * 512],
                            kT[po:po + 64, m, c2 * 128:(c2 + 1) * 128],
                            qT[po:po + 64, m, hf * 512:(hf + 1) * 512],
                            start=True, stop=True,
                        )
                    nc.scalar.activation(expt[:, c2, :], ps, EXPFN, scale=0.125)
                ctx = cx_ps.tile([DH + 1, S], F32, tag="cx")
                for c2 in range(SC):
                    for hf in range(2):
                        nc.tensor.matmul(
                            ctx[:, hf * 512:(hf + 1) * 512],
                            Vaug[:, c2, h, :],
                            expt[:, c2, hf * 512:(hf + 1) * 512],
                            start=(c2 == 0), stop=(c2 == SC - 1),
                        )
                srow = sm_pool.tile([1, S], F32, tag="srow")
                nc.vector.tensor_copy(srow, ctx[DH:DH + 1, :])
                rrow = sm_pool.tile([1, S], F32, tag="rrow")
                nc.vector.reciprocal_approx_fast(rrow, srow)
                rr = rr_pool.tile([64, S], F32, tag="rr")
                nc.gpsimd.partition_broadcast(rr, rrow)
                t = t_pool.tile([64, S], F32, tag="t")
                nc.vector.scalar_tensor_tensor(
                    t, ctx[0:DH, :], wscale, rr,
                    op0=AluOpType.mult, op1=AluOpType.mult)
                return t

            for h in range(NH):
                po = 64 * (h % 2)
                m = h // 2
                # group A: shares KT / Vp stationaries
                t1 = branch(KT, QT, Vp, h, w11)    # stream-x self
                t4 = branch(KT, QdT, Vp, h, w22)   # stream-y cross
                # group B: shares KdT / Vdp
                t2 = branch(KdT, QdT, Vdp, h, w21)  # stream-y self
                t3 = branch(KdT, QT, Vdp, h, w12)   # stream-x cross
                nc.vector.tensor_add(CX[po:po + 64, m, :], t1, t3)
                nc.vector.tensor_add(CY[po:po + 64, m, :], t2, t4)

        # ---------------- Phase 3: output projections ----------------
        with ExitStack() as ph:
            op_ps = ph.enter_context(tc.tile_pool(name="opps", bufs=4, space="PSUM"))
            ob = ph.enter_context(tc.tile_pool(name="ob", bufs=3))
            wo_pool = ph.enter_context(tc.tile_pool(name="wo", bufs=1))

            WoT = wo_pool.tile([128, HC, H], BF16, tag="WoT")
            WodT = wo_pool.tile([128, HC, H], BF16, tag="WodT")
            nc.sync.dma_start(WoT, wdram("WoT"))
            nc.sync.dma_start(WodT, wdram("WodT"))

            out_r = out_d.rearrange("o (c p) s -> o p c s", p=128)
            for st, (Wt, CT, bn) in enumerate(
                    ((WoT, CX, "bfx"), (WodT, CY, "bfy"))):
                for m in range(HC):
                    for sn in range(2):
                        ps = op_ps.tile([128, 512], F32, tag="op")
                        for c in range(HC):
                            nc.tensor.matmul(
                                ps,
                                Wt[:, c, m * 128:(m + 1) * 128],
                                CT[:, c, sn * 512:(sn + 1) * 512],
                                start=(c == 0), stop=(c == HC - 1),
                            )
                        ot = ob.tile([128, 512], F32, tag="ot")
                        if bn in bias_t:
                            nc.vector.tensor_scalar_add(
                                ot, ps, bias_t[bn][:, m:m + 1])
                        else:
                            nc.vector.tensor_copy(ot, ps)
                        nc.sync.dma_start(
                            out_r[st, :, m, sn * 512:(sn + 1) * 512], ot)

    nc.compile()
    return nc


_CACHE = {}


def _get_nc(scalars, bias_flags):
    key = (scalars, tuple(sorted(bias_flags.items())))
    if key not in _CACHE:
        _CACHE[key] = _build(scalars, bias_flags)
    return _CACHE[key]


def _prep(inputs):
    """Host-side prep: shard, transpose, cast. Returns (in_maps, scalars, flags)."""
    f = {k: np.asarray(v) for k, v in inputs.items()}
    hx = f["hidden_statesx"].astype(np.float32)
    hy = f["hidden_statesy"].astype(np.float32)
    scalars = tuple(float(np.asarray(f[k]).reshape(-1)[0])
                    for k in ("w11", "w12", "w21", "w22"))
    w11, w12, w21, w22 = scalars

    def bf(a):
        return np.ascontiguousarray(a).astype(ml_dtypes.bfloat16)

    shared = {}
    for name, key in (("WqT", "Wq"), ("WkT", "Wk"), ("WvT", "Wv"),
                      ("WqdT", "Wqd"), ("WkdT", "Wkd"), ("WvdT", "Wvd"),
                      ("WoT", "Wo"), ("WodT", "Wod")):
        shared[name] = bf(f[key].astype(np.float32).T)

    # bias folding: bv/bo-type biases fold into one per-partition bias on the
    # final output (probs rows sum to 1 => probs @ (v + bv) = probs@v + bv).
    Wo = f["Wo"].astype(np.float64)
    Wod = f["Wod"].astype(np.float64)
    bv = f["bv"].astype(np.float64)
    bvd = f["bvd"].astype(np.float64)
    bfx = Wo @ (w11 * bv + w12 * bvd) + (w11 + w12) * f["bo"].astype(np.float64)
    bfy = Wod @ (w21 * bvd + w22 * bv) + (w21 + w22) * f["bod"].astype(np.float64)
    bias_arrs = {
        "bq": f["bq"].astype(np.float32), "bk": f["bk"].astype(np.float32),
        "bqd": f["bqd"].astype(np.float32), "bkd": f["bkd"].astype(np.float32),
        "bfx": bfx.astype(np.float32), "bfy": bfy.astype(np.float32),
    }
    flags = {}
    for n, a in bias_arrs.items():
        if np.any(a != 0.0):
            flags[n] = True
            shared[n] = a

    in_maps = []
    for c in range(N_CORES):
        m = dict(shared)
        m["xT"] = bf(hx[c].T)
        m["yT"] = bf(hy[c].T)
        in_maps.append(m)
    return in_maps, scalars, flags


def _run(inputs, trace=False):
    in_maps, scalars, flags = _prep(inputs)
    nc = _get_nc(scalars, flags)
    res = run_bass_kernel_spmd(nc, in_maps, core_ids=list(range(N_CORES)),
                               trace=trace)
    outs = np.stack([res.results[c]["out"] for c in range(N_CORES)])  # [8,2,H,S]
    sx = np.ascontiguousarray(outs[:, 0].transpose(0, 2, 1))  # [8, S, H]
    sy = np.ascontiguousarray(outs[:, 1].transpose(0, 2, 1))
    return (sx, sy), res


def kernel(**inputs):
    (sx, sy), _ = _run(inputs, trace=False)
    return sx, sy


# revision 32
# speedup vs baseline: 2.8719x; 2.8719x over previous
"""Trainium2 Bass kernel: dual-stream multi-branch attention (batch-parallel over 8 cores).

Reference computation (per batch element b):
    q  = x @ Wq.T + bq ; k = x @ Wk.T + bk ; v = x @ Wv.T + bv      (x stream)
    qd = y @ Wqd.T+ bqd; kd = y @ Wkd.T+bkd; vd = y @ Wvd.T+bvd     (y stream)
    4 attention branches over 12 heads (DH=64):
        b1 = attn(q, k, v)    -> stream-x self    (weight w11, proj Wo)
        b2 = attn(qd, kd, vd) -> stream-y self    (w21, Wod)
        b3 = attn(q, kd, vd)  -> stream-x cross   (w12, Wo)
        b4 = attn(qd, k, v)   -> stream-y cross   (w22, Wod)
    out_x = (w11*ctx1 + w12*ctx3) @ Wo.T  + (w11+w12)*bo
    out_y = (w21*ctx2 + w22*ctx4) @ Wod.T + (w21+w22)*bod

Device design (per core = one batch element, SPMD on 8 cores, no collectives):
  - Activations kept TRANSPOSED in SBUF: xT/yT/QT/KT [768(e-chunks of 128 partitions), 1024].
  - V kept NATURAL [s, e], augmented with a ones column per head -> softmax
    denominators come free as row 64 of the ctx matmul output.
  - scoresT[s2, s1] = k_h @ q_h.T via matmul(lhsT=kT_h, rhs=qT_h); exp on ACT
    (scale=1/8 folded into ACT's free affine); ctxT[d, s1] accumulated over s2
    chunks with lhsT = [v_h | ones].
  - Head PAIRS (2h, 2h+1) are processed with row-tiled concurrent matmuls
    (tile_position auto-derived from partition bases 0/64) - scores run at
    full PE-array utilization.
  - Software pipeline: unit = (pair, group, branch); each unit's scores+exp
    are interleaved in issue order with the PREVIOUS unit's ctx matmuls so
    the PE fills its ACT-wait gaps. PSUM: 2 score tiles (4 banks) + 2 ctx
    accumulators (4 banks) rotate.
  - Q/K projections are emitted per-e-chunk (m), immediately followed by that
    pair's attention units, so ACT exp work starts ~60us into the kernel.
  - Normalization: rr = 1/sums broadcast over partitions (gpsimd), fused
    scale+mul on DVE; merged per stream into CTXm.T (bf16) = out-proj rhs.
  - All matmul operands bf16 (PSUM accumulates f32).
"""

import numpy as np
import ml_dtypes
from contextlib import ExitStack

import concourse.bass as bass
import concourse.mybir as mybir
import concourse.tile as tile
from concourse import bacc
from concourse.alu_op_type import AluOpType
from concourse.bass_utils import run_bass_kernel_spmd

B, S, H = 8, 1024, 768
NH, DH = 12, 64
HC = H // 128          # 6 chunks of 128 along hidden dim
SC = S // 128          # 8 chunks of 128 along sequence dim
N_CORES = 8
BF16 = mybir.dt.bfloat16
F32 = mybir.dt.float32
EXPFN = mybir.ActivationFunctionType.Exp


def _build(scalars, bias_flags, body_reps=1):
    """Build the Bass program. scalars = (w11, w12, w21, w22) python floats
    (baked as immediates). bias_flags: dict name->bool for nonzero biases.
    body_reps>1 repeats the whole compute body (benchmark calibration only)."""
    w11, w12, w21, w22 = scalars
    nc = bacc.Bacc(debug=False)

    def din(name, shape, dt=BF16):
        return nc.declare_dram_parameter(name, list(shape), dt, isOutput=False)

    xT_d = din("xT", (H, S))
    yT_d = din("yT", (H, S))
    wd = {n: din(n, (H, H)) for n in
          ("WqT", "WkT", "WvT", "WqdT", "WkdT", "WvdT", "WoT", "WodT")}
    bias_d = {}
    for n in ("bq", "bk", "bqd", "bkd", "bfx", "bfy"):
        if bias_flags.get(n):
            bias_d[n] = din(n, (H,), F32)
    out_d = nc.declare_dram_parameter("out", [2, H, S], F32, isOutput=True)

    def wdram(n):
        # [768, 768] -> [128 partitions, chunk, 768]
        return wd[n].rearrange("(c p) e -> p c e", p=128)

    with tile.TileContext(nc) as tc:
      for _rep in range(body_reps):
        _build_body(nc, tc, _rep, scalars, xT_d, yT_d, wdram, bias_d, out_d)

    nc.compile()
    return nc


def _build_body(nc, tc, _rep, scalars, xT_d, yT_d, wdram, bias_d, out_d):
    w11, w12, w21, w22 = scalars
    with ExitStack() as top:
        P = lambda name, bufs, **kw: top.enter_context(
            tc.tile_pool(name=f"{name}{_rep}", bufs=bufs, **kw))

        qk_pool = P("qk", 2)      # per-m rotating Q/K slices
        v_pool = P("vv", 1)
        ctxm_pool = P("ctxm", 1)
        bias_pool = P("bias", 1)
        xy = P("xy", 1)
        wrot = P("wrot", 2)
        wqk = P("wqk", 4)
        ob_pool = P("ob", 3)
        sc_ps = P("scps", 2, space="PSUM")
        cx_ps = P("cxps", 2, space="PSUM")
        ex_pool = P("exp", 4)
        sm_pool = P("small", 2)
        rr_pool = P("rrp", 2)
        t_pool = P("tmrg", 2)

        Vp = v_pool.tile([128, SC, NH, DH + 1], BF16, tag="Vp")
        Vdp = v_pool.tile([128, SC, NH, DH + 1], BF16, tag="Vdp")
        CX = [ctxm_pool.tile([128, S], BF16, tag=f"CX{m}", name=f"CX{m}")
              for m in range(HC)]
        CY = [ctxm_pool.tile([128, S], BF16, tag=f"CY{m}", name=f"CY{m}")
              for m in range(HC)]

        nc.gpsimd.memset(Vp[:, :, :, DH:DH + 1], 1.0)
        nc.gpsimd.memset(Vdp[:, :, :, DH:DH + 1], 1.0)

        bias_t = {}
        for n in ("bq", "bk", "bqd", "bkd", "bfx", "bfy"):
            if n in bias_d:
                t = bias_pool.tile([128, HC], F32, tag=n)
                nc.sync.dma_start(t, bias_d[n].rearrange("(c p) -> p c", p=128))
                bias_t[n] = t

        xT = xy.tile([128, HC, S], BF16, tag="xT")
        yT = xy.tile([128, HC, S], BF16, tag="yT")
        # split input DMAs so the first projection matmuls start early
        for half in range(4):
            sl = slice(half * 256, (half + 1) * 256)
            nc.sync.dma_start(
                xT[:, :, sl],
                xT_d.rearrange("(c p) s -> p c s", p=128)[:, :, sl])
            nc.sync.dma_start(
                yT[:, :, sl],
                yT_d.rearrange("(c p) s -> p c s", p=128)[:, :, sl])

        # ---- V / Vd projections (natural layout, ones-augmented) ----
        # en=0 (heads 0-5) emitted upfront; en=1 (heads 6-11, not needed until
        # pair 3) returned as deferred series to spread across early pairs.
        def v_proj_series(en):
            series = []
            for wname, dst, act in (("WvT", Vp, xT), ("WvdT", Vdp, yT)):
                W = wrot.tile([128, HC, 384], BF16, tag="Wv",
                              name=f"Wv{en}")
                nc.sync.dma_start(
                    W, wdram(wname)[:, :, en * 384:(en + 1) * 384])
                for s in range(SC):
                    def emit(s=s, W=W, act=act, dst=dst, en=en):
                        ps = cx_ps.tile([128, 6, DH], F32, tag="cx", name="pv")
                        for c in range(HC):
                            nc.tensor.matmul(
                                ps,
                                act[:, c, s * 128:(s + 1) * 128],
                                W[:, c, :],
                                start=(c == 0), stop=(c == HC - 1),
                            )
                        nc.vector.tensor_copy(
                            dst[:, s, en * 6:(en + 1) * 6, 0:DH], ps)
                    series.append(emit)
            return series

        v0_series = v_proj_series(0)
        v1_series = v_proj_series(1)

        # ---- per-m-chunk Q/K projections (rotating [128, S] slices) ----
        def proj_qk_plan(m):
            """Allocate dst tiles + W DMAs for e-chunk m; return (qk, series)
            where series is 8 deferred emitters (one PSUM accumulation each)
            to be spread across the previous pair's attention units."""
            qk = {}
            series = []
            for wname, tag, act, bn in (
                ("WqT", "QT", xT, "bq"), ("WkT", "KT", xT, "bk"),
                ("WqdT", "QdT", yT, "bqd"), ("WkdT", "KdT", yT, "bkd"),
            ):
                dst = qk_pool.tile([128, S], BF16, tag=tag, name=tag + "t")
                qk[tag] = dst
                W = wqk.tile([128, HC, 128], BF16, tag="Wqk", name="Wqk")
                nc.sync.dma_start(
                    W, wdram(wname)[:, :, m * 128:(m + 1) * 128])

                def emit(sn, W=W, act=act, dst=dst, bn=bn):
                    ps = sc_ps.tile([128, 512], F32, tag="sc", name="pq")
                    for c in range(HC):
                        nc.tensor.matmul(
                            ps,
                            W[:, c, :],
                            act[:, c, sn * 512:(sn + 1) * 512],
                            start=(c == 0), stop=(c == HC - 1),
                        )
                    dslice = dst[:, sn * 512:(sn + 1) * 512]
                    if bn in bias_t:
                        nc.vector.tensor_scalar_add(
                            dslice, ps, bias_t[bn][:, m:m + 1])
                    else:
                        nc.vector.tensor_copy(dslice, ps)

                series.append(lambda e=emit: e(0))
                series.append(lambda e=emit: e(1))
            return qk, series

        # ---- attention units: unit = (pair m, group, branch) covering heads
        # h0=2m, h1=2m+1. Software-pipelined: emit_scores(unit) interleaved
        # with emit_ctx(prev unit).
        # group A: kT=KT, Vaug=Vp;  branch s: qT=QT  (b1 -> CX, w11)
        #                           branch c: qT=QdT (b4 -> CY, w22)
        # group B: kT=KdT, Vaug=Vdp; branch s: qT=QdT (b2 -> CY, w21)
        #                            branch c: qT=QT  (b3 -> CX, w12)
        def unit_specs(m, qk):
            for g in ("A", "B"):
                kT = qk["KT"] if g == "A" else qk["KdT"]
                Vaug = Vp if g == "A" else Vdp
                for br in ("s", "c"):
                    if g == "A":
                        qT = qk["QT"] if br == "s" else qk["QdT"]
                        dstCT, wsc = (CX, w11) if br == "s" else (CY, w22)
                    else:
                        qT = qk["QdT"] if br == "s" else qk["QT"]
                        dstCT, wsc = (CY, w21) if br == "s" else (CX, w12)
                    yield (m, g, br, kT, qT, Vaug, dstCT, wsc)

        def emit_scores_chunk(u, c2):
            """Row-tiled concurrent scores for heads 2m / 2m+1, chunk c2;
            then exp both. Returns nothing (exp tiles tracked in u)."""
            (m, g, br, kT, qT, Vaug, dstCT, wsc) = u["spec"]
            for hi in range(2):
                po = 64 * hi
                ps = sc_ps.tile([128, S], F32, tag="sc")
                for hf in range(2):
                    nc.tensor.matmul(
                        ps[:, hf * 512:(hf + 1) * 512],
                        kT[po:po + 64, c2 * 128:(c2 + 1) * 128],
                        qT[po:po + 64, hf * 512:(hf + 1) * 512],
                        start=True, stop=True,
                    )
                nc.scalar.activation(u["expt"][hi][:, c2, :], ps, EXPFN,
                                     scale=0.125)

        def emit_ctx_chunk(u, c2):
            (m, g, br, kT, qT, Vaug, dstCT, wsc) = u["spec"]
            for hi in range(2):
                h = 2 * m + hi
                for hf in range(2):
                    nc.tensor.matmul(
                        u["ctx"][hi][:, hf * 512:(hf + 1) * 512],
                        Vaug[:, c2, h, :],
                        u["expt"][hi][:, c2, hf * 512:(hf + 1) * 512],
                        start=(c2 == 0), stop=(c2 == SC - 1),
                    )

        def emit_normalize(u):
            """sums -> rr -> normalized scaled ctxT -> merge into CX/CY."""
            (m, g, br, kT, qT, Vaug, dstCT, wsc) = u["spec"]
            for hi in range(2):
                po = 64 * hi
                ctx = u["ctx"][hi]
                srow = sm_pool.tile([1, S], F32, tag="srow")
                nc.vector.tensor_copy(srow, ctx[DH:DH + 1, :])
                rrow = sm_pool.tile([1, S], F32, tag="rrow")
                nc.vector.reciprocal_approx_fast(rrow, srow)
                rr = rr_pool.tile([64, S], F32, tag="rr")
                nc.gpsimd.partition_broadcast(rr, rrow)
                dslice = dstCT[m][po:po + 64, :]
                if g == "A":
                    # group A units write CX/CY first: dst = (ctx*w) * rr
                    nc.vector.scalar_tensor_tensor(
                        dslice, ctx[0:DH, :], wsc, rr,
                        op0=AluOpType.mult, op1=AluOpType.mult)
                else:
                    # group B units accumulate: dst += (ctx*w) * rr
                    # (t sliced at po so tensor_add inputs share a base
                    # partition - walrus requires it for SB+SB inputs)
                    t = t_pool.tile([128, S], F32, tag="t")
                    nc.vector.scalar_tensor_tensor(
                        t[po:po + 64, :], ctx[0:DH, :], wsc, rr,
                        op0=AluOpType.mult, op1=AluOpType.mult)
                    nc.vector.tensor_add(dslice, dslice, t[po:po + 64, :])

        # ---- output projection series (weights DMA'd as per-m slices) ----
        out_r = out_d.rearrange("o (c p) s -> o p c s", p=128)

        def outproj_series(st, wname, CT, bn):
            series = []
            for m in range(HC):
                Wsl = wqk.tile([128, HC, 128], BF16, tag="Wqk",
                               name=f"Wo{st}m{m}")
                nc.sync.dma_start(
                    Wsl, wdram(wname)[:, :, m * 128:(m + 1) * 128])
                for sn in range(2):
                    def emit(m=m, sn=sn, Wsl=Wsl, CT=CT, bn=bn, st=st):
                        ps = sc_ps.tile([128, 512], F32, tag="sc", name="op")
                        for c in range(HC):
                            nc.tensor.matmul(
                                ps,
                                Wsl[:, c, :],
                                CT[c][:, sn * 512:(sn + 1) * 512],
                                start=(c == 0), stop=(c == HC - 1),
                            )
                        ot = ob_pool.tile([128, 512], F32, tag="ot",
                                          name="ot")
                        if bn in bias_t:
                            nc.vector.tensor_scalar_add(
                                ot, ps, bias_t[bn][:, m:m + 1])
                        else:
                            nc.vector.tensor_copy(ot, ps)
                        nc.sync.dma_start(
                            out_r[st, :, m, sn * 512:(sn + 1) * 512], ot)
                    series.append(emit)
            return series

        # ---- main software-pipelined emission ----
        from collections import deque
        fillers = deque()
        prev = None
        qk, _series0 = proj_qk_plan(0)
        for s_emit in _series0:
            s_emit()
        v0 = deque(v0_series)
        fillers.extend(v1_series)
        for m in range(HC):
            if m + 1 < HC:
                qk_next, nxt = proj_qk_plan(m + 1)
                # next pair's projections are urgent (consumed by pair m+1's
                # first scores) - they go to the FRONT of the filler queue
                fillers = deque(nxt) + fillers
            else:
                qk_next = None
            for ui, spec in enumerate(unit_specs(m, qk)):
                u = {
                    "spec": spec,
                    "expt": [ex_pool.tile([128, SC, S], BF16, tag="exp",
                                          name=f"expt{hi}")
                             for hi in range(2)],
                    "ctx": None,
                }
                # per chunk: ready ctx work FIRST (PE is in-order; a stalled
                # scores matmul must not block ready work behind it), fillers
                # mid-loop, then this unit's scores+exp.
                for c2 in range(SC):
                    if prev is not None:
                        emit_ctx_chunk(prev, c2)
                    if v0:
                        for _ in range(2):
                            if v0:
                                v0.popleft()()
                    elif c2 in (3, 6):
                        for _ in range(2):
                            if fillers:
                                fillers.popleft()()
                    emit_scores_chunk(u, c2)
                if prev is not None:
                    emit_normalize(prev)
                u["ctx"] = [cx_ps.tile([DH + 1, S], F32, tag="cx",
                                       name=f"ctx{hi}")
                            for hi in range(2)]
                prev = u
            qk = qk_next
        # drain last unit (pair-5 B-c), then its normalize BEFORE any output
        # copies queue on DVE (the final CX chunk gates stream-x outproj);
        # stream-y outproj (CY complete) runs while that normalize drains.
        for c2 in range(SC):
            emit_ctx_chunk(prev, c2)
        emit_normalize(prev)
        for s_emit in outproj_series(1, "WodT", CY, "bfy"):
            s_emit()
        for s_emit in outproj_series(0, "WoT", CX, "bfx"):
            s_emit()


_CACHE = {}


def _get_nc(scalars, bias_flags, body_reps=1):
    key = (scalars, tuple(sorted(bias_flags.items())), body_reps)
    if key not in _CACHE:
        _CACHE[key] = _build(scalars, bias_flags, body_reps=body_reps)
    return _CACHE[key]


def _prep(inputs):
    """Host-side prep: shard, transpose, cast. Returns (in_maps, scalars, flags)."""
    f = {k: np.asarray(v) for k, v in inputs.items()}
    hx = f["hidden_statesx"].astype(np.float32)
    hy = f["hidden_statesy"].astype(np.float32)
    scalars = tuple(float(np.asarray(f[k]).reshape(-1)[0])
                    for k in ("w11", "w12", "w21", "w22"))
    w11, w12, w21, w22 = scalars

    def bf(a):
        return np.ascontiguousarray(a).astype(ml_dtypes.bfloat16)

    shared = {}
    for name, key in (("WqT", "Wq"), ("WkT", "Wk"), ("WvT", "Wv"),
                      ("WqdT", "Wqd"), ("WkdT", "Wkd"), ("WvdT", "Wvd"),
                      ("WoT", "Wo"), ("WodT", "Wod")):
        shared[name] = bf(f[key].astype(np.float32).T)

    # bias folding: bv/bo-type biases fold into one per-partition bias on the
    # final output (probs rows sum to 1 => probs @ (v + bv) = probs@v + bv).
    Wo = f["Wo"].astype(np.float64)
    Wod = f["Wod"].astype(np.float64)
    bv = f["bv"].astype(np.float64)
    bvd = f["bvd"].astype(np.float64)
    bfx = Wo @ (w11 * bv + w12 * bvd) + (w11 + w12) * f["bo"].astype(np.float64)
    bfy = Wod @ (w21 * bvd + w22 * bv) + (w21 + w22) * f["bod"].astype(np.float64)
    bias_arrs = {
        "bq": f["bq"].astype(np.float32), "bk": f["bk"].astype(np.float32),
        "bqd": f["bqd"].astype(np.float32), "bkd": f["bkd"].astype(np.float32),
        "bfx": bfx.astype(np.float32), "bfy": bfy.astype(np.float32),
    }
    flags = {}
    for n, a in bias_arrs.items():
        if np.any(a != 0.0):
            flags[n] = True
            shared[n] = a

    in_maps = []
    for c in range(N_CORES):
        m = dict(shared)
        m["xT"] = bf(hx[c].T)
        m["yT"] = bf(hy[c].T)
        in_maps.append(m)
    return in_maps, scalars, flags


def _run(inputs, trace=False):
    in_maps, scalars, flags = _prep(inputs)
    nc = _get_nc(scalars, flags)
    res = run_bass_kernel_spmd(nc, in_maps, core_ids=list(range(N_CORES)),
                               trace=trace)
    outs = np.stack([res.results[c]["out"] for c in range(N_CORES)])  # [8,2,H,S]
    sx = np.ascontiguousarray(outs[:, 0].transpose(0, 2, 1))  # [8, S, H]
    sy = np.ascontiguousarray(outs[:, 1].transpose(0, 2, 1))
    return (sx, sy), res


def kernel(**inputs):
    (sx, sy), _ = _run(inputs, trace=False)
    return sx, sy


# revision 45
# speedup vs baseline: 3.1257x; 1.0884x over previous
"""Trainium2 Bass kernel: dual-stream multi-branch attention (batch-parallel over 8 cores).

Reference computation (per batch element b):
    q  = x @ Wq.T + bq ; k = x @ Wk.T + bk ; v = x @ Wv.T + bv      (x stream)
    qd = y @ Wqd.T+ bqd; kd = y @ Wkd.T+bkd; vd = y @ Wvd.T+bvd     (y stream)
    4 attention branches over 12 heads (DH=64):
        b1 = attn(q, k, v)    -> stream-x self    (weight w11, proj Wo)
        b2 = attn(qd, kd, vd) -> stream-y self    (w21, Wod)
        b3 = attn(q, kd, vd)  -> stream-x cross   (w12, Wo)
        b4 = attn(qd, k, v)   -> stream-y cross   (w22, Wod)
    out_x = (w11*ctx1 + w12*ctx3) @ Wo.T  + (w11+w12)*bo
    out_y = (w21*ctx2 + w22*ctx4) @ Wod.T + (w21+w22)*bod

Device design (per core = one batch element, SPMD on 8 cores, no collectives):
  - Activations kept TRANSPOSED in SBUF: xT/yT/QT/KT [768(e-chunks of 128 partitions), 1024].
  - V kept NATURAL [s, e], augmented with a ones column per head -> softmax
    denominators come free as row 64 of the ctx matmul output.
  - scoresT[s2, s1] = k_h @ q_h.T via matmul(lhsT=kT_h, rhs=qT_h); exp on ACT
    (scale=1/8 folded into ACT's free affine); ctxT[d, s1] accumulated over s2
    chunks with lhsT = [v_h | ones].
  - Head PAIRS (2h, 2h+1) are processed with row-tiled concurrent matmuls
    (tile_position auto-derived from partition bases 0/64) - scores run at
    full PE-array utilization.
  - Software pipeline: unit = (pair, group, branch); each unit's scores+exp
    are interleaved in issue order with the PREVIOUS unit's ctx matmuls so
    the PE fills its ACT-wait gaps. PSUM: 2 score tiles (4 banks) + 2 ctx
    accumulators (4 banks) rotate.
  - Q/K projections are emitted per-e-chunk (m), immediately followed by that
    pair's attention units, so ACT exp work starts ~60us into the kernel.
  - Normalization: rr = 1/sums broadcast over partitions (gpsimd), fused
    scale+mul on DVE; merged per stream into CTXm.T (bf16) = out-proj rhs.
  - All matmul operands bf16 (PSUM accumulates f32).
"""

import numpy as np
import ml_dtypes
from contextlib import ExitStack

import concourse.bass as bass
import concourse.mybir as mybir
import concourse.tile as tile
from concourse import bacc
from concourse.alu_op_type import AluOpType
from concourse.bass_utils import run_bass_kernel_spmd

B, S, H = 8, 1024, 768
NH, DH = 12, 64
HC = H // 128          # 6 chunks of 128 along hidden dim
SC = S // 128          # 8 chunks of 128 along sequence dim
N_CORES = 8
BF16 = mybir.dt.bfloat16
F32 = mybir.dt.float32
EXPFN = mybir.ActivationFunctionType.Exp


def _build(scalars, bias_flags, body_reps=1):
    """Build the Bass program. scalars = (w11, w12, w21, w22) python floats
    (baked as immediates). bias_flags: dict name->bool for nonzero biases.
    body_reps>1 repeats the whole compute body (benchmark calibration only)."""
    w11, w12, w21, w22 = scalars
    nc = bacc.Bacc(debug=False)

    def din(name, shape, dt=BF16):
        return nc.declare_dram_parameter(name, list(shape), dt, isOutput=False)

    xT_d = din("xT", (H, S))
    yT_d = din("yT", (H, S))
    wd = {n: din(n, (H, H)) for n in
          ("WqT", "WkT", "WvT", "WqdT", "WkdT", "WvdT", "WoT", "WodT")}
    bias_d = {}
    for n in ("bq", "bk", "bqd", "bkd", "bfx", "bfy"):
        if bias_flags.get(n):
            bias_d[n] = din(n, (H,), F32)
    out_d = nc.declare_dram_parameter("out", [2, H, S], F32, isOutput=True)

    def wdram(n):
        # [768, 768] -> [128 partitions, chunk, 768]
        return wd[n].rearrange("(c p) e -> p c e", p=128)

    with tile.TileContext(nc) as tc:
      for _rep in range(body_reps):
        _build_body(nc, tc, _rep, scalars, xT_d, yT_d, wdram, bias_d, out_d)

    nc.compile()
    return nc


def _build_body(nc, tc, _rep, scalars, xT_d, yT_d, wdram, bias_d, out_d):
    w11, w12, w21, w22 = scalars
    with ExitStack() as top:
        P = lambda name, bufs, **kw: top.enter_context(
            tc.tile_pool(name=f"{name}{_rep}", bufs=bufs, **kw))

        qk_pool = P("qk", 2)      # per-m rotating Q/K slices
        v_pool = P("vv", 1)
        ctxm_pool = P("ctxm", 1)
        bias_pool = P("bias", 1)
        xy = P("xy", 1)
        wrot = P("wrot", 2)
        wqk = P("wqk", 4)
        ob_pool = P("ob", 3)
        sc_ps = P("scps", 2, space="PSUM")
        cx_ps = P("cxps", 2, space="PSUM")
        ex_pool = P("exp", 4)
        sm_pool = P("small", 2)
        rr_pool = P("rrp", 2)
        t_pool = P("tmrg", 2)

        Vp = v_pool.tile([128, SC, NH, DH + 1], BF16, tag="Vp")
        Vdp = v_pool.tile([128, SC, NH, DH + 1], BF16, tag="Vdp")
        CX = [ctxm_pool.tile([128, S], BF16, tag=f"CX{m}", name=f"CX{m}")
              for m in range(HC)]
        CY = [ctxm_pool.tile([128, S], BF16, tag=f"CY{m}", name=f"CY{m}")
              for m in range(HC)]

        nc.gpsimd.memset(Vp[:, :, :, DH:DH + 1], 1.0)
        nc.gpsimd.memset(Vdp[:, :, :, DH:DH + 1], 1.0)

        bias_t = {}
        for n in ("bq", "bk", "bqd", "bkd", "bfx", "bfy"):
            if n in bias_d:
                t = bias_pool.tile([128, HC], F32, tag=n)
                nc.sync.dma_start(t, bias_d[n].rearrange("(c p) -> p c", p=128))
                bias_t[n] = t

        xT = xy.tile([128, HC, S], BF16, tag="xT")
        yT = xy.tile([128, HC, S], BF16, tag="yT")
        # split input DMAs so the first projection matmuls start early
        for half in range(4):
            sl = slice(half * 256, (half + 1) * 256)
            nc.sync.dma_start(
                xT[:, :, sl],
                xT_d.rearrange("(c p) s -> p c s", p=128)[:, :, sl])
            nc.sync.dma_start(
                yT[:, :, sl],
                yT_d.rearrange("(c p) s -> p c s", p=128)[:, :, sl])

        # ---- V / Vd projections (natural layout, ones-augmented) ----
        # en=0 (heads 0-5) emitted upfront; en=1 (heads 6-11, not needed until
        # pair 3) returned as deferred series to spread across early pairs.
        def v_proj_series(en):
            series = []
            for wname, dst, act in (("WvT", Vp, xT), ("WvdT", Vdp, yT)):
                W = wrot.tile([128, HC, 384], BF16, tag="Wv",
                              name=f"Wv{en}")
                nc.sync.dma_start(
                    W, wdram(wname)[:, :, en * 384:(en + 1) * 384])
                for s in range(SC):
                    def emit(s=s, W=W, act=act, dst=dst, en=en):
                        ps = cx_ps.tile([128, 6, DH], F32, tag="cx", name="pv")
                        for c in range(HC):
                            nc.tensor.matmul(
                                ps,
                                act[:, c, s * 128:(s + 1) * 128],
                                W[:, c, :],
                                start=(c == 0), stop=(c == HC - 1),
                            )
                        nc.vector.tensor_copy(
                            dst[:, s, en * 6:(en + 1) * 6, 0:DH], ps)
                    series.append(emit)
            return series

        v0_series = v_proj_series(0)
        v1_series = v_proj_series(1)

        # ---- per-m-chunk Q/K projections (rotating [128, S] slices) ----
        def proj_qk_plan(m):
            """Allocate dst tiles + W DMAs for e-chunk m; return (qk, series)
            where series is 8 deferred emitters (one PSUM accumulation each)
            to be spread across the previous pair's attention units."""
            qk = {}
            series = []
            for wname, tag, act, bn in (
                ("WqT", "QT", xT, "bq"), ("WkT", "KT", xT, "bk"),
                ("WqdT", "QdT", yT, "bqd"), ("WkdT", "KdT", yT, "bkd"),
            ):
                dst = qk_pool.tile([128, S], BF16, tag=tag, name=tag + "t")
                qk[tag] = dst
                W = wqk.tile([128, HC, 128], BF16, tag="Wqk", name="Wqk")
                nc.sync.dma_start(
                    W, wdram(wname)[:, :, m * 128:(m + 1) * 128])

                def emit(sn, W=W, act=act, dst=dst, bn=bn):
                    ps = sc_ps.tile([128, 512], F32, tag="sc", name="pq")
                    for c in range(HC):
                        nc.tensor.matmul(
                            ps,
                            W[:, c, :],
                            act[:, c, sn * 512:(sn + 1) * 512],
                            start=(c == 0), stop=(c == HC - 1),
                        )
                    dslice = dst[:, sn * 512:(sn + 1) * 512]
                    if bn in bias_t:
                        nc.vector.tensor_scalar_add(
                            dslice, ps, bias_t[bn][:, m:m + 1])
                    else:
                        nc.vector.tensor_copy(dslice, ps)

                series.append(lambda e=emit: e(0))
                series.append(lambda e=emit: e(1))
            return qk, series

        # ---- attention units: unit = (pair m, group, branch) covering heads
        # h0=2m, h1=2m+1. Software-pipelined: emit_scores(unit) interleaved
        # with emit_ctx(prev unit).
        # group A: kT=KT, Vaug=Vp;  branch s: qT=QT  (b1 -> CX, w11)
        #                           branch c: qT=QdT (b4 -> CY, w22)
        # group B: kT=KdT, Vaug=Vdp; branch s: qT=QdT (b2 -> CY, w21)
        #                            branch c: qT=QT  (b3 -> CX, w12)
        def unit_specs(m, qk):
            for g in ("A", "B"):
                kT = qk["KT"] if g == "A" else qk["KdT"]
                Vaug = Vp if g == "A" else Vdp
                for br in ("s", "c"):
                    if g == "A":
                        qT = qk["QT"] if br == "s" else qk["QdT"]
                        dstCT, wsc = (CX, w11) if br == "s" else (CY, w22)
                    else:
                        qT = qk["QdT"] if br == "s" else qk["QT"]
                        dstCT, wsc = (CY, w21) if br == "s" else (CX, w12)
                    yield (m, g, br, kT, qT, Vaug, dstCT, wsc)

        def emit_scores_chunk(u, c2):
            """Row-tiled concurrent scores for heads 2m / 2m+1, chunk c2;
            then exp both. Returns nothing (exp tiles tracked in u)."""
            (m, g, br, kT, qT, Vaug, dstCT, wsc) = u["spec"]
            for hi in range(2):
                po = 64 * hi
                ps = sc_ps.tile([128, S], F32, tag="sc")
                for hf in range(2):
                    nc.tensor.matmul(
                        ps[:, hf * 512:(hf + 1) * 512],
                        kT[po:po + 64, c2 * 128:(c2 + 1) * 128],
                        qT[po:po + 64, hf * 512:(hf + 1) * 512],
                        start=True, stop=True,
                    )
                nc.scalar.activation(u["expt"][hi][:, c2, :], ps, EXPFN,
                                     scale=0.125)

        def emit_ctx_chunk(u, c2):
            (m, g, br, kT, qT, Vaug, dstCT, wsc) = u["spec"]
            for hi in range(2):
                h = 2 * m + hi
                for hf in range(2):
                    nc.tensor.matmul(
                        u["ctx"][hi][:, hf * 512:(hf + 1) * 512],
                        Vaug[:, c2, h, :],
                        u["expt"][hi][:, c2, hf * 512:(hf + 1) * 512],
                        start=(c2 == 0), stop=(c2 == SC - 1),
                    )

        def emit_normalize(u):
            """sums -> rr -> normalized scaled ctxT -> merge into CX/CY."""
            (m, g, br, kT, qT, Vaug, dstCT, wsc) = u["spec"]
            for hi in range(2):
                po = 64 * hi
                ctx = u["ctx"][hi]
                srow = sm_pool.tile([1, S], F32, tag="srow")
                nc.vector.tensor_copy(srow, ctx[DH:DH + 1, :])
                rrow = sm_pool.tile([1, S], F32, tag="rrow")
                nc.vector.reciprocal_approx_fast(rrow, srow)
                rr = rr_pool.tile([64, S], F32, tag="rr")
                nc.gpsimd.partition_broadcast(rr, rrow)
                dslice = dstCT[m][po:po + 64, :]
                if g == "A":
                    # group A units write CX/CY first: dst = (ctx*w) * rr
                    nc.vector.scalar_tensor_tensor(
                        dslice, ctx[0:DH, :], wsc, rr,
                        op0=AluOpType.mult, op1=AluOpType.mult)
                else:
                    # group B units accumulate: dst += (ctx*w) * rr
                    # (t sliced at po so tensor_add inputs share a base
                    # partition - walrus requires it for SB+SB inputs)
                    t = t_pool.tile([128, S], F32, tag="t")
                    nc.vector.scalar_tensor_tensor(
                        t[po:po + 64, :], ctx[0:DH, :], wsc, rr,
                        op0=AluOpType.mult, op1=AluOpType.mult)
                    nc.vector.tensor_add(dslice, dslice, t[po:po + 64, :])

        # ---- output projection series (weights DMA'd as per-m slices) ----
        out_r = out_d.rearrange("o (c p) s -> o p c s", p=128)

        def outproj_series(st, wname, CT, bn):
            series = []
            for m in range(HC):
                Wsl = wqk.tile([128, HC, 128], BF16, tag="Wqk",
                               name=f"Wo{st}m{m}")
                nc.sync.dma_start(
                    Wsl, wdram(wname)[:, :, m * 128:(m + 1) * 128])
                for sn in range(2):
                    def emit(m=m, sn=sn, Wsl=Wsl, CT=CT, bn=bn, st=st):
                        ps = sc_ps.tile([128, 512], F32, tag="sc", name="op")
                        for c in range(HC):
                            nc.tensor.matmul(
                                ps,
                                Wsl[:, c, :],
                                CT[c][:, sn * 512:(sn + 1) * 512],
                                start=(c == 0), stop=(c == HC - 1),
                            )
                        ot = ob_pool.tile([128, 512], F32, tag="ot",
                                          name="ot")
                        if bn in bias_t:
                            nc.vector.tensor_scalar_add(
                                ot, ps, bias_t[bn][:, m:m + 1])
                        else:
                            nc.vector.tensor_copy(ot, ps)
                        nc.sync.dma_start(
                            out_r[st, :, m, sn * 512:(sn + 1) * 512], ot)
                    series.append(emit)
            return series

        # ---- main software-pipelined emission ----
        from collections import deque
        fillers = deque()
        prev = None
        qk, _series0 = proj_qk_plan(0)
        # pair-0 unit A-s only needs QT/KT: emit those 4 series now, defer
        # QdT/KdT (needed from unit A-c) to the front of the in-loop queue
        for s_emit in _series0[:4]:
            s_emit()
        v0 = deque(_series0[4:] + v0_series)
        fillers.extend(v1_series)
        for m in range(HC):
            if m + 1 < HC:
                qk_next, nxt = proj_qk_plan(m + 1)
                # next pair's projections are urgent (consumed by pair m+1's
                # first scores) - they go to the FRONT of the filler queue
                fillers = deque(nxt) + fillers
            else:
                qk_next = None
            for ui, spec in enumerate(unit_specs(m, qk)):
                u = {
                    "spec": spec,
                    "expt": [ex_pool.tile([128, SC, S], BF16, tag="exp",
                                          name=f"expt{hi}")
                             for hi in range(2)],
                    "ctx": None,
                }
                # per chunk: ready ctx work FIRST (PE is in-order; a stalled
                # scores matmul must not block ready work behind it), fillers
                # mid-loop, then this unit's scores+exp.
                for c2 in range(SC):
                    if prev is not None:
                        emit_ctx_chunk(prev, c2)
                    if v0:
                        for _ in range(2):
                            if v0:
                                v0.popleft()()
                    elif c2 in (3, 6):
                        for _ in range(2):
                            if fillers:
                                fillers.popleft()()
                    emit_scores_chunk(u, c2)
                if prev is not None:
                    emit_normalize(prev)
                u["ctx"] = [cx_ps.tile([DH + 1, S], F32, tag="cx",
                                       name=f"ctx{hi}")
                            for hi in range(2)]
                prev = u
            qk = qk_next
        # drain last unit (pair-5 B-c), then its normalize BEFORE any output
        # copies queue on DVE (the final CX chunk gates stream-x outproj);
        # stream-y outproj (CY complete) runs while that normalize drains.
        for c2 in range(SC):
            emit_ctx_chunk(prev, c2)
        emit_normalize(prev)
        for s_emit in outproj_series(1, "WodT", CY, "bfy"):
            s_emit()
        for s_emit in outproj_series(0, "WoT", CX, "bfx"):
            s_emit()


_CACHE = {}


def _get_nc(scalars, bias_flags, body_reps=1):
    key = (scalars, tuple(sorted(bias_flags.items())), body_reps)
    if key not in _CACHE:
        _CACHE[key] = _build(scalars, bias_flags, body_reps=body_reps)
    return _CACHE[key]


def _prep(inputs):
    """Host-side prep: shard, transpose, cast. Returns (in_maps, scalars, flags)."""
    f = {k: np.asarray(v) for k, v in inputs.items()}
    hx = f["hidden_statesx"].astype(np.float32)
    hy = f["hidden_statesy"].astype(np.float32)
    scalars = tuple(float(np.asarray(f[k]).reshape(-1)[0])
                    for k in ("w11", "w12", "w21", "w22"))
    w11, w12, w21, w22 = scalars

    def bf(a):
        return np.ascontiguousarray(a).astype(ml_dtypes.bfloat16)

    shared = {}
    for name, key in (("WqT", "Wq"), ("WkT", "Wk"), ("WvT", "Wv"),
                      ("WqdT", "Wqd"), ("WkdT", "Wkd"), ("WvdT", "Wvd"),
                      ("WoT", "Wo"), ("WodT", "Wod")):
        shared[name] = bf(f[key].astype(np.float32).T)

    # bias folding: bv/bo-type biases fold into one per-partition bias on the
    # final output (probs rows sum to 1 => probs @ (v + bv) = probs@v + bv).
    Wo = f["Wo"].astype(np.float64)
    Wod = f["Wod"].astype(np.float64)
    bv = f["bv"].astype(np.float64)
    bvd = f["bvd"].astype(np.float64)
    bfx = Wo @ (w11 * bv + w12 * bvd) + (w11 + w12) * f["bo"].astype(np.float64)
    bfy = Wod @ (w21 * bvd + w22 * bv) + (w21 + w22) * f["bod"].astype(np.float64)
    bias_arrs = {
        "bq": f["bq"].astype(np.float32), "bk": f["bk"].astype(np.float32),
        "bqd": f["bqd"].astype(np.float32), "bkd": f["bkd"].astype(np.float32),
        "bfx": bfx.astype(np.float32), "bfy": bfy.astype(np.float32),
    }
    flags = {}
    for n, a in bias_arrs.items():
        if np.any(a != 0.0):
            flags[n] = True
            shared[n] = a

    in_maps = []
    for c in range(N_CORES):
        m = dict(shared)
        m["xT"] = bf(hx[c].T)
        m["yT"] = bf(hy[c].T)
        in_maps.append(m)
    return in_maps, scalars, flags


def _run(inputs, trace=False):
    in_maps, scalars, flags = _prep(inputs)
    nc = _get_nc(scalars, flags)
    res = run_bass_kernel_spmd(nc, in_maps, core_ids=list(range(N_CORES)),
                               trace=trace)
    outs = np.stack([res.results[c]["out"] for c in range(N_CORES)])  # [8,2,H,S]
    sx = np.ascontiguousarray(outs[:, 0].transpose(0, 2, 1))  # [8, S, H]
    sy = np.ascontiguousarray(outs[:, 1].transpose(0, 2, 1))
    return (sx, sy), res


def kernel(**inputs):
    (sx, sy), _ = _run(inputs, trace=False)
    return sx, sy
